# revision 8
# baseline (speedup 1.0000x reference)
"""Compressed Interaction Network (CIN) kernel for Trainium2, 8 NeuronCores.

Reference computation (per layer l with weights W[F0, Fk, S], bias b[S]):
    z[b,s,d] = relu( sum_{h,k} x0[b,h,d] * xk[b,k,d] * W[h,k,s] + b[s] )
    split_half: xk_next = z[:, :S/2, :], direct_l = z[:, S/2:, :] (last: all)
    out = sum_d concat(direct_0, direct_1, direct_2)    # [B, 64+64+128]

Strategy (v2 — fp16, DRAM-streamed replication):
  - Data parallel over batch: each of 8 cores gets B/8 = 256 batches,
    working in transposed layout [field, bd], bd = b*16 + d (BD = 4096).
  - Per layer, flatten (h, k) h-major into 128-row chunks. The moving
    matmul operand p[(h,k), bd] = x0[h,bd] * xk[k,bd] is built on DVE /
    GpSimd as (replicated x0) * (stacked xk), all in fp16 (DVE 2x mode).
  - The replicated-x0 factor is precomputed on the HOST and streamed from
    DRAM in fp16 with fat (16KB/partition) descriptors — no on-chip
    broadcast DMAs, no replication matmuls. Layers 1 and 2 share the same
    replication pattern, so their rep tiles are loaded once and held in
    SBUF across both layers.
  - bd is split into 2 half-BD groups of 2048 cols (4 bd-tiles of 512).
    Per group: L0 (13 chunks x 117 rows) -> L1 -> L2 (20 chunks x 128).
    z accumulates in PSUM fp32 (8 banks = 2 groups x 4 tiles); fp16
    matmuls run at 1 cycle/row at any PE p-state.
  - Epilogue: ScalarE relu+bias -> fp16 (xk halves + direct tmp), one fat
    SBUF copy duplicates the stacked xk half, DVE reduces over d.
"""
import numpy as np

import concourse.bass as bass
import concourse.mybir as mybir
from concourse.tile import TileContext
from concourse.bass_utils import run_bass_kernel_spmd

F32 = mybir.dt.float32
F16 = mybir.dt.float16
MULT = mybir.AluOpType.mult
ADD = mybir.AluOpType.add
RELU = mybir.ActivationFunctionType.Relu
AXX = mybir.AxisListType.X

N_CORES = 8
B, F0, D = 2048, 39, 16
S = 128                     # layer size
BC = B // N_CORES           # 256 batches per core
BD = BC * D                 # 4096 columns per core
NGRP = 2                    # half-BD groups
GW = BD // NGRP             # 2048 cols per group
NT = 512                    # bd-tile width (PSUM bank)
TPG = GW // NT              # 4 tiles per group
L0_CH, L0_P = 13, 117       # layer-0: 13 chunks of 117 = 3h x 39k
L12_CH = 20                 # layers 1/2: 19 full 128-chunks + one 64-chunk
BND = 4                     # rep chunks per DMA bundle
GPS_L0 = (5, 11)            # chunk ids multiplied on GpSimd
GPS_L12 = (4, 9, 14, 19)

MAX_WAITS = 1


def _fix_sync_overflow(nc):
    """This walrus build accepts at most one semaphore wait per instruction;
    Tile can attach several. Hoist extras onto NoOps spliced right before the
    offending instruction on the same engine (same-engine order is
    sequential, so earlier waits are equivalent). Updates stay put."""
    n_new = 0
    for blk in nc.main_func.blocks:
        out = []
        changed = False
        for inst in blk.instructions:
            si = inst.sync_info
            waits = list(si.on_wait) if si is not None else []
            if len(waits) > MAX_WAITS:
                changed = True
                extra, keep = waits[:-MAX_WAITS], waits[-MAX_WAITS:]
                for i in range(0, len(extra), MAX_WAITS):
                    nop = mybir.InstNoOp(name=f"wsplit-{n_new}", ins=[], outs=[])
                    n_new += 1
                    nop.engine = inst.engine
                    nop.sync_info = mybir.SyncInfo(
                        on_wait=extra[i:i + MAX_WAITS], on_update=[])
                    nc.register_instruction(nop, overwrite=True)
                    out.append(nop)
                si.on_wait = keep
            out.append(inst)
        if changed:
            blk.instructions = out
    return n_new


def _build_kernel():
    nc = bass.Bass(trn_type="TRN2")

    rep0 = nc.dram_tensor("rep0", [L0_P, NGRP, L0_CH, GW], F16,
                          kind="ExternalInput")
    rep12 = nc.dram_tensor("rep12", [S, NGRP, L12_CH, GW], F16,
                           kind="ExternalInput")
    x0st3 = nc.dram_tensor("x0st3", [L0_P, BD], F16, kind="ExternalInput")
    w0 = nc.dram_tensor("w0", [L0_P, L0_CH * S], F16, kind="ExternalInput")
    w1 = nc.dram_tensor("w1", [S, L12_CH * S], F16, kind="ExternalInput")
    w2 = nc.dram_tensor("w2", [S, L12_CH * S], F16, kind="ExternalInput")
    biases = nc.dram_tensor("biases", [S, 4], F32, kind="ExternalInput")
    y = nc.dram_tensor("y", [2 * S, BC], F32, kind="ExternalOutput")

    with TileContext(nc) as tc:
        with tc.tile_pool(name="static", bufs=1) as st, \
             tc.tile_pool(name="rep", bufs=7) as rp, \
             tc.tile_pool(name="p", bufs=4) as pp, \
             tc.tile_pool(name="tmp", bufs=3) as tp, \
             tc.tile_pool(name="zps", bufs=8, space="PSUM") as zp:

            # ---- static tiles -------------------------------------------
            x0st3_s = st.tile([L0_P, BD], F16)
            xk1_s = st.tile([S, BD], F16)
            xk2_s = st.tile([S, BD], F16)
            w0_s = st.tile([L0_P, L0_CH * S], F16)
            w1_s = st.tile([S, L12_CH * S], F16)
            w2_s = st.tile([S, L12_CH * S], F16)
            bias_s = st.tile([S, 4], F32)
            o0_s = st.tile([S, BC], F32)
            o1_s = st.tile([S, BC], F32)
            o2_s = st.tile([S, BC], F32)

            nc.sync.dma_start(x0st3_s[:, :], x0st3[:, :])
            nc.scalar.dma_start(w0_s[:, :], w0[:, :])
            nc.gpsimd.dma_start(bias_s[:, :], biases[:, :])

            # round-robin rep loads over the two HWDGE queues (SP, Act) and
            # the Pool SWDGE queue — per-queue load processing is serial, so
            # one queue alone bottlenecks the stream
            dma_eng = [nc.sync, nc.scalar, nc.gpsimd]
            qctr = [0]

            def next_q():
                q = dma_eng[qctr[0] % len(dma_eng)]
                qctr[0] += 1
                return q

            def load_bundle(g, l, bi):
                """Allocate + DMA one rep bundle (l in {0, 1})."""
                c = bi * BND
                nch = L0_CH if l == 0 else L12_CH
                part_full = L0_P if l == 0 else S
                nb = min(BND, nch - c)
                bundle = rp.tile([S, BND * GW], F16, tag="rep",
                                 name=f"bundle{g}{l}{bi}")
                src = rep0 if l == 0 else rep12
                next_q().dma_start(bundle[:part_full, 0:nb * GW],
                                   src[0:part_full, g, c:c + nb, :])
                return bundle

            def layer_gen(g, l, rep_hold, xk_next, odst, bias_col, pre=()):
                """Emit one layer for group g, yielding after each chunk."""
                gof = g * GW
                nch = L0_CH if l == 0 else L12_CH
                part_full = L0_P if l == 0 else S
                in0 = x0st3_s if l == 0 else (xk1_s if l == 1 else xk2_s)
                wt = w0_s if l == 0 else (w1_s if l == 1 else w2_s)
                gps_set = GPS_L0 if l == 0 else GPS_L12
                zs = [zp.tile([S, NT], F32, tag="z", name=f"z{g}{l}{t}")
                      for t in range(TPG)]
                bundle = None
                for c in range(nch):
                    part = 64 if (l > 0 and c == nch - 1) else part_full
                    bi, ci = divmod(c, BND)
                    if ci == 0:
                        if l == 2:
                            bundle = rep_hold[bi]
                        else:
                            bundle = pre[bi] if bi < len(pre) \
                                else load_bundle(g, l, bi)
                            if l == 1:
                                rep_hold.append(bundle)
                    rep_ap = bundle[:part, ci * GW:(ci + 1) * GW]
                    p = pp.tile([S, GW], F16, tag="p")
                    eng = nc.gpsimd if c in gps_set else nc.vector
                    eng.tensor_tensor(p[:part, :], in0[:part, gof:gof + GW],
                                      rep_ap, op=MULT)
                    for t in range(TPG):
                        nc.tensor.matmul(
                            zs[t][:, :], wt[:part, bass.ts(c, S)],
                            p[:part, bass.ts(t, NT)],
                            start=(c == 0), stop=(c == nch - 1))
                    yield
                # epilogue: bias + relu -> fp16; xk halves; direct reduce
                for t in range(TPG):
                    ts = bass.ts(g * TPG + t, NT)
                    ocol = bass.ts(g * TPG + t, NT // D)
                    bias_ap = bias_s[:, bias_col:bias_col + 1]
                    if xk_next is not None:
                        nc.scalar.activation(
                            xk_next[0:64, ts], zs[t][0:64, :], RELU,
                            bias=bias_s[0:64, bias_col:bias_col + 1])
                        tmp = tp.tile([S, NT], F16, tag="tmp")
                        nc.scalar.activation(
                            tmp[64:S, :], zs[t][64:S, :], RELU,
                            bias=bias_s[64:S, bias_col:bias_col + 1])
                        nc.vector.tensor_reduce(
                            odst[64:S, ocol],
                            tmp[64:S, :].rearrange("p (b d) -> p b d", d=D),
                            axis=AXX, op=ADD)
                    else:
                        tmp = tp.tile([S, NT], F16, tag="tmp")
                        nc.scalar.activation(tmp[:, :], zs[t][:, :], RELU,
                                             bias=bias_ap)
                        nc.vector.tensor_reduce(
                            odst[:, ocol],
                            tmp[:, :].rearrange("p (b d) -> p b d", d=D),
                            axis=AXX, op=ADD)
                if xk_next is not None:
                    # duplicate the stacked xk half with one fat SBUF copy
                    dma_eng[g % 2].dma_start(
                        xk_next[64:S, gof:gof + GW],
                        xk_next[0:64, gof:gof + GW])
                yield

            def run(*gens):
                gens = list(gens)
                while gens:
                    for gen in list(gens):
                        try:
                            next(gen)
                        except StopIteration:
                            gens.remove(gen)

            # Sequential phases with one interleave zone (L2(A) || L1(B)).
            # All tile-pool ring-slot reuses point backward in trace order,
            # which keeps the per-engine in-order queues deadlock-free.
            repA, repB = [], []
            run(layer_gen(0, 0, None, xk1_s, o0_s, 0))
            nc.sync.dma_start(w1_s[:, :], w1[:, :])
            run(layer_gen(1, 0, None, xk1_s, o0_s, 0))
            nc.scalar.dma_start(w2_s[:, :], w2[:, :])
            run(layer_gen(0, 1, repA, xk2_s, o1_s, 1))
            # prefetch B's first two L1 bundles (free ring slots) so the
            # interleave zone doesn't stall on just-in-time loads
            preB = [load_bundle(1, 1, 0), load_bundle(1, 1, 1)]
            run(layer_gen(0, 2, repA, None, o2_s, 2),
                layer_gen(1, 1, repB, xk2_s, o1_s, 1, pre=preB))
            run(layer_gen(1, 2, repB, None, o2_s, 2))

            nc.sync.dma_start(y[0:64, :], o0_s[64:S, :])
            nc.scalar.dma_start(y[64:S, :], o1_s[64:S, :])
            nc.sync.dma_start(y[S:2 * S, :], o2_s[:, :])

    _fix_sync_overflow(nc)
    return nc


_NC_CACHE = None


def _get_nc():
    global _NC_CACHE
    if _NC_CACHE is None:
        _NC_CACHE = _build_kernel()
    return _NC_CACHE


def _prep_core_inputs(x16, w_list, b_list, core):
    """Host-side layout prep for one core's batch slice. x16: [F0, B*D] f16
    full-batch transposed input."""
    x0t = x16[:, core * BD:(core + 1) * BD]          # [39, 4096] f16

    # rep0[(j*39+k), g, c, col] = x0t[3c+j, g*2048+col]
    a = np.ascontiguousarray(x0t).reshape(L0_CH, 3, NGRP, GW)
    rep0 = np.broadcast_to(a[:, :, None, :, :],
                           (L0_CH, 3, F0, NGRP, GW))
    rep0 = np.ascontiguousarray(rep0.transpose(1, 2, 3, 0, 4)) \
        .reshape(L0_P, NGRP, L0_CH, GW)

    # rep12[(j*64+k), g, c, col] = x0t[2c+j, g*2048+col]  (h=39 row zero)
    xp = np.concatenate([x0t, np.zeros((1, BD), np.float16)], axis=0)
    a = xp.reshape(L12_CH, 2, NGRP, GW)
    rep12 = np.broadcast_to(a[:, :, None, :, :],
                            (L12_CH, 2, 64, NGRP, GW))
    rep12 = np.ascontiguousarray(rep12.transpose(1, 2, 3, 0, 4)) \
        .reshape(S, NGRP, L12_CH, GW)

    x0st3 = np.ascontiguousarray(np.tile(x0t, (3, 1)))   # [117, 4096]

    return {"rep0": rep0, "rep12": rep12, "x0st3": x0st3,
            "w0": w_list[0], "w1": w_list[1], "w2": w_list[2],
            "biases": b_list}


def kernel(inputs, w0, w1, w2, b0, b1, b2, _trace=False):
    inputs = np.asarray(inputs, np.float32)
    x16 = np.ascontiguousarray(
        inputs.transpose(1, 0, 2).reshape(F0, B * D)).astype(np.float16)

    w0f = np.asarray(w0, np.float32)
    w1f = np.asarray(w1, np.float32)
    w2f = np.asarray(w2, np.float32)
    # w0c[(j*39+k), c*128:+128] = w0f[3c+j, k, :]
    w0c = np.ascontiguousarray(
        w0f.reshape(L0_CH, 3, F0, S).transpose(1, 2, 0, 3)
        .reshape(L0_P, L0_CH * S)).astype(np.float16)
    wc12 = []
    for wf in (w1f, w2f):
        wp = np.concatenate([wf, np.zeros((1, 64, S), np.float32)], axis=0)
        wc = np.ascontiguousarray(
            wp.reshape(L12_CH, 2, 64, S).transpose(1, 2, 0, 3)
            .reshape(S, L12_CH * S)).astype(np.float16)
        wc12.append(wc)
    w_list = [w0c, wc12[0], wc12[1]]

    bmat = np.zeros((S, 4), np.float32)
    bmat[:, 0] = np.asarray(b0, np.float32)
    bmat[:, 1] = np.asarray(b1, np.float32)
    bmat[:, 2] = np.asarray(b2, np.float32)

    nc = _get_nc()
    in_maps = [_prep_core_inputs(x16, w_list, bmat, core)
               for core in range(N_CORES)]
    res = run_bass_kernel_spmd(nc, in_maps, core_ids=list(range(N_CORES)),
                               trace=_trace)
    outs = []
    for core in range(N_CORES):
        yc = res.results[core]["y"]          # [256 s_cat, 256 b]
        outs.append(np.ascontiguousarray(yc.T))
    full = np.concatenate(outs, axis=0)       # [2048, 256]
    if _trace:
        return full, res
    return full


# revision 13
# speedup vs baseline: 1.0630x; 1.0630x over previous
"""Compressed Interaction Network (CIN) kernel for Trainium2, 8 NeuronCores.

Reference computation (per layer l with weights W[F0, Fk, S], bias b[S]):
    z[b,s,d] = relu( sum_{h,k} x0[b,h,d] * xk[b,k,d] * W[h,k,s] + b[s] )
    split_half: xk_next = z[:, :S/2, :], direct_l = z[:, S/2:, :] (last: all)
    out = sum_d concat(direct_0, direct_1, direct_2)    # [B, 64+64+128]

Strategy (v2 — fp16, DRAM-streamed replication):
  - Data parallel over batch: each of 8 cores gets B/8 = 256 batches,
    working in transposed layout [field, bd], bd = b*16 + d (BD = 4096).
  - Per layer, flatten (h, k) h-major into 128-row chunks. The moving
    matmul operand p[(h,k), bd] = x0[h,bd] * xk[k,bd] is built on DVE /
    GpSimd as (replicated x0) * (stacked xk), all in fp16 (DVE 2x mode).
  - The replicated-x0 factor is precomputed on the HOST and streamed from
    DRAM in fp16 with fat (16KB/partition) descriptors — no on-chip
    broadcast DMAs, no replication matmuls. Layers 1 and 2 share the same
    replication pattern, so their rep tiles are loaded once and held in
    SBUF across both layers.
  - bd is split into 2 half-BD groups of 2048 cols (4 bd-tiles of 512).
    Per group: L0 (13 chunks x 117 rows) -> L1 -> L2 (20 chunks x 128).
    z accumulates in PSUM fp32 (8 banks = 2 groups x 4 tiles); fp16
    matmuls run at 1 cycle/row at any PE p-state.
  - Epilogue: ScalarE relu+bias -> fp16 (xk halves + direct tmp), one fat
    SBUF copy duplicates the stacked xk half, DVE reduces over d.
"""
import numpy as np

import concourse.bass as bass
import concourse.mybir as mybir
from concourse.tile import TileContext
from concourse.bass_utils import run_bass_kernel_spmd

F32 = mybir.dt.float32
F16 = mybir.dt.float16
MULT = mybir.AluOpType.mult
ADD = mybir.AluOpType.add
RELU = mybir.ActivationFunctionType.Relu
AXX = mybir.AxisListType.X

N_CORES = 8
B, F0, D = 2048, 39, 16
S = 128                     # layer size
BC = B // N_CORES           # 256 batches per core
BD = BC * D                 # 4096 columns per core
NGRP = 2                    # half-BD groups
GW = BD // NGRP             # 2048 cols per group
NT = 512                    # bd-tile width (PSUM bank)
TPG = GW // NT              # 4 tiles per group
L0_CH, L0_P = 13, 117       # layer-0: 13 chunks of 117 = 3h x 39k
L12_CH = 20                 # layers 1/2: 19 full 128-chunks + one 64-chunk
BND = 2                     # rep chunks per DMA bundle
GPS_L0 = (5, 11)            # chunk ids multiplied on GpSimd
GPS_L12 = (4, 9, 14, 19)

MAX_WAITS = 1


def _fix_sync_overflow(nc):
    """This walrus build accepts at most one semaphore wait per instruction;
    Tile can attach several. Hoist extras onto NoOps spliced right before the
    offending instruction on the same engine (same-engine order is
    sequential, so earlier waits are equivalent). Updates stay put."""
    n_new = 0
    for blk in nc.main_func.blocks:
        out = []
        changed = False
        for inst in blk.instructions:
            si = inst.sync_info
            waits = list(si.on_wait) if si is not None else []
            if len(waits) > MAX_WAITS:
                changed = True
                extra, keep = waits[:-MAX_WAITS], waits[-MAX_WAITS:]
                for i in range(0, len(extra), MAX_WAITS):
                    nop = mybir.InstNoOp(name=f"wsplit-{n_new}", ins=[], outs=[])
                    n_new += 1
                    nop.engine = inst.engine
                    nop.sync_info = mybir.SyncInfo(
                        on_wait=extra[i:i + MAX_WAITS], on_update=[])
                    nc.register_instruction(nop, overwrite=True)
                    out.append(nop)
                si.on_wait = keep
            out.append(inst)
        if changed:
            blk.instructions = out
    return n_new


def _build_kernel():
    nc = bass.Bass(trn_type="TRN2")

    rep0 = nc.dram_tensor("rep0", [L0_P, NGRP, L0_CH, GW], F16,
                          kind="ExternalInput")
    rep12 = nc.dram_tensor("rep12", [S, NGRP, L12_CH, GW], F16,
                           kind="ExternalInput")
    x0st3 = nc.dram_tensor("x0st3", [L0_P, BD], F16, kind="ExternalInput")
    w0 = nc.dram_tensor("w0", [L0_P, L0_CH * S], F16, kind="ExternalInput")
    w1 = nc.dram_tensor("w1", [S, L12_CH * S], F16, kind="ExternalInput")
    w2 = nc.dram_tensor("w2", [S, L12_CH * S], F16, kind="ExternalInput")
    biases = nc.dram_tensor("biases", [S, 4], F32, kind="ExternalInput")
    y = nc.dram_tensor("y", [2 * S, BC], F32, kind="ExternalOutput")

    with TileContext(nc) as tc:
        with tc.tile_pool(name="static", bufs=1) as st, \
             tc.tile_pool(name="rep", bufs=15) as rp, \
             tc.tile_pool(name="p", bufs=4) as pp, \
             tc.tile_pool(name="tmp", bufs=3) as tp, \
             tc.tile_pool(name="zps", bufs=8, space="PSUM") as zp:

            # ---- static tiles -------------------------------------------
            x0st3_s = st.tile([L0_P, BD], F16)
            xk1_s = st.tile([S, BD], F16)
            xk2_s = st.tile([S, BD], F16)
            w0_s = st.tile([L0_P, L0_CH * S], F16)
            w1_s = st.tile([S, L12_CH * S], F16)
            w2_s = st.tile([S, L12_CH * S], F16)
            bias_s = st.tile([S, 4], F32)
            o0_s = st.tile([S, BC], F32)
            o1_s = st.tile([S, BC], F32)
            o2_s = st.tile([S, BC], F32)

            # startup: only x0st3's first half, w0, bias and the first rep
            # bundle gate the first multiply — give each its own queue
            nc.sync.dma_start(x0st3_s[:, 0:GW], x0st3[:, 0:GW])
            nc.gpsimd.dma_start(w0_s[:, :], w0[:, :])
            nc.gpsimd.dma_start(bias_s[:, :], biases[:, :])

            # round-robin rep loads over the two HWDGE queues (SP, Act) and
            # the Pool SWDGE queue — per-queue load processing is serial, so
            # one queue alone bottlenecks the stream
            dma_eng = [nc.scalar, nc.sync, nc.gpsimd]
            qctr = [0]

            def next_q():
                q = dma_eng[qctr[0] % len(dma_eng)]
                qctr[0] += 1
                return q

            def load_bundle(g, l, bi):
                """Allocate + DMA one rep bundle (l in {0, 1})."""
                c = bi * BND
                nch = L0_CH if l == 0 else L12_CH
                part_full = L0_P if l == 0 else S
                nb = min(BND, nch - c)
                bundle = rp.tile([S, BND * GW], F16, tag="rep",
                                 name=f"bundle{g}{l}{bi}")
                src = rep0 if l == 0 else rep12
                next_q().dma_start(bundle[:part_full, 0:nb * GW],
                                   src[0:part_full, g, c:c + nb, :])
                return bundle

            def layer_gen(g, l, rep_hold, xk_next, odst, bias_col, pre=()):
                """Emit one layer for group g, yielding after each chunk."""
                gof = g * GW
                nch = L0_CH if l == 0 else L12_CH
                part_full = L0_P if l == 0 else S
                in0 = x0st3_s if l == 0 else (xk1_s if l == 1 else xk2_s)
                wt = w0_s if l == 0 else (w1_s if l == 1 else w2_s)
                gps_set = GPS_L0 if l == 0 else GPS_L12
                zs = [zp.tile([S, NT], F32, tag="z", name=f"z{g}{l}{t}")
                      for t in range(TPG)]
                bundle = None
                for c in range(nch):
                    part = 64 if (l > 0 and c == nch - 1) else part_full
                    bi, ci = divmod(c, BND)
                    if ci == 0:
                        if l == 2:
                            bundle = rep_hold[bi]
                        else:
                            bundle = pre[bi] if bi < len(pre) \
                                else load_bundle(g, l, bi)
                            if l == 1:
                                rep_hold.append(bundle)
                    rep_ap = bundle[:part, ci * GW:(ci + 1) * GW]
                    p = pp.tile([S, GW], F16, tag="p")
                    eng = nc.gpsimd if c in gps_set else nc.vector
                    eng.tensor_tensor(p[:part, :], in0[:part, gof:gof + GW],
                                      rep_ap, op=MULT)
                    for t in range(TPG):
                        nc.tensor.matmul(
                            zs[t][:, :], wt[:part, bass.ts(c, S)],
                            p[:part, bass.ts(t, NT)],
                            start=(c == 0), stop=(c == nch - 1))
                    yield
                # epilogue: bias + relu -> fp16; xk halves; direct reduce
                for t in range(TPG):
                    ts = bass.ts(g * TPG + t, NT)
                    ocol = bass.ts(g * TPG + t, NT // D)
                    bias_ap = bias_s[:, bias_col:bias_col + 1]
                    if xk_next is not None:
                        nc.scalar.activation(
                            xk_next[0:64, ts], zs[t][0:64, :], RELU,
                            bias=bias_s[0:64, bias_col:bias_col + 1])
                        tmp = tp.tile([S, NT], F16, tag="tmp")
                        nc.scalar.activation(
                            tmp[64:S, :], zs[t][64:S, :], RELU,
                            bias=bias_s[64:S, bias_col:bias_col + 1])
                        nc.vector.tensor_reduce(
                            odst[64:S, ocol],
                            tmp[64:S, :].rearrange("p (b d) -> p b d", d=D),
                            axis=AXX, op=ADD)
                    else:
                        tmp = tp.tile([S, NT], F16, tag="tmp")
                        nc.scalar.activation(tmp[:, :], zs[t][:, :], RELU,
                                             bias=bias_ap)
                        nc.vector.tensor_reduce(
                            odst[:, ocol],
                            tmp[:, :].rearrange("p (b d) -> p b d", d=D),
                            axis=AXX, op=ADD)
                if xk_next is not None:
                    # duplicate the stacked xk half with one fat SBUF copy
                    dma_eng[g % 2].dma_start(
                        xk_next[64:S, gof:gof + GW],
                        xk_next[0:64, gof:gof + GW])
                # store this group's finished output rows right away
                ocb = g * (BC // NGRP)
                oce = ocb + BC // NGRP
                if l == 2:
                    dma_eng[(g + 1) % 2].dma_start(
                        y[S:2 * S, ocb:oce], odst[:, ocb:oce])
                else:
                    dma_eng[(g + 1) % 2].dma_start(
                        y[l * 64:(l + 1) * 64, ocb:oce],
                        odst[64:S, ocb:oce])
                yield

            def run(*gens):
                gens = list(gens)
                while gens:
                    for gen in list(gens):
                        try:
                            next(gen)
                        except StopIteration:
                            gens.remove(gen)

            # Sequential phases with one interleave zone (L2(A) || L1(B)).
            # All tile-pool ring-slot reuses point backward in trace order,
            # which keeps the per-engine in-order queues deadlock-free.
            repA, repB = [], []
            genA0 = layer_gen(0, 0, None, xk1_s, o0_s, 0)
            next(genA0)           # emit A-L0 chunk 0 before the bulk loads
            nc.sync.dma_start(x0st3_s[:, GW:BD], x0st3[:, GW:BD])
            run(genA0)
            nc.sync.dma_start(w1_s[:, :], w1[:, :])
            run(layer_gen(1, 0, None, xk1_s, o0_s, 0))
            nc.scalar.dma_start(w2_s[:, :], w2[:, :])
            run(layer_gen(0, 1, repA, xk2_s, o1_s, 1))
            # prefetch B's first two L1 bundles (free ring slots) so the
            # interleave zone doesn't stall on just-in-time loads
            preB = [load_bundle(1, 1, 0), load_bundle(1, 1, 1)]
            run(layer_gen(0, 2, repA, None, o2_s, 2),
                layer_gen(1, 1, repB, xk2_s, o1_s, 1, pre=preB))
            run(layer_gen(1, 2, repB, None, o2_s, 2))

    _fix_sync_overflow(nc)
    return nc


_NC_CACHE = None


def _get_nc():
    global _NC_CACHE
    if _NC_CACHE is None:
        _NC_CACHE = _build_kernel()
    return _NC_CACHE


def _prep_core_inputs(x16, w_list, b_list, core):
    """Host-side layout prep for one core's batch slice. x16: [F0, B*D] f16
    full-batch transposed input."""
    x0t = x16[:, core * BD:(core + 1) * BD]          # [39, 4096] f16

    # rep0[(j*39+k), g, c, col] = x0t[3c+j, g*2048+col]
    a = np.ascontiguousarray(x0t).reshape(L0_CH, 3, NGRP, GW)
    rep0 = np.broadcast_to(a[:, :, None, :, :],
                           (L0_CH, 3, F0, NGRP, GW))
    rep0 = np.ascontiguousarray(rep0.transpose(1, 2, 3, 0, 4)) \
        .reshape(L0_P, NGRP, L0_CH, GW)

    # rep12[(j*64+k), g, c, col] = x0t[2c+j, g*2048+col]  (h=39 row zero)
    xp = np.concatenate([x0t, np.zeros((1, BD), np.float16)], axis=0)
    a = xp.reshape(L12_CH, 2, NGRP, GW)
    rep12 = np.broadcast_to(a[:, :, None, :, :],
                            (L12_CH, 2, 64, NGRP, GW))
    rep12 = np.ascontiguousarray(rep12.transpose(1, 2, 3, 0, 4)) \
        .reshape(S, NGRP, L12_CH, GW)

    x0st3 = np.ascontiguousarray(np.tile(x0t, (3, 1)))   # [117, 4096]

    return {"rep0": rep0, "rep12": rep12, "x0st3": x0st3,
            "w0": w_list[0], "w1": w_list[1], "w2": w_list[2],
            "biases": b_list}


def kernel(inputs, w0, w1, w2, b0, b1, b2, _trace=False):
    inputs = np.asarray(inputs, np.float32)
    x16 = np.ascontiguousarray(
        inputs.transpose(1, 0, 2).reshape(F0, B * D)).astype(np.float16)

    w0f = np.asarray(w0, np.float32)
    w1f = np.asarray(w1, np.float32)
    w2f = np.asarray(w2, np.float32)
    # w0c[(j*39+k), c*128:+128] = w0f[3c+j, k, :]
    w0c = np.ascontiguousarray(
        w0f.reshape(L0_CH, 3, F0, S).transpose(1, 2, 0, 3)
        .reshape(L0_P, L0_CH * S)).astype(np.float16)
    wc12 = []
    for wf in (w1f, w2f):
        wp = np.concatenate([wf, np.zeros((1, 64, S), np.float32)], axis=0)
        wc = np.ascontiguousarray(
            wp.reshape(L12_CH, 2, 64, S).transpose(1, 2, 0, 3)
            .reshape(S, L12_CH * S)).astype(np.float16)
        wc12.append(wc)
    w_list = [w0c, wc12[0], wc12[1]]

    bmat = np.zeros((S, 4), np.float32)
    bmat[:, 0] = np.asarray(b0, np.float32)
    bmat[:, 1] = np.asarray(b1, np.float32)
    bmat[:, 2] = np.asarray(b2, np.float32)

    nc = _get_nc()
    in_maps = [_prep_core_inputs(x16, w_list, bmat, core)
               for core in range(N_CORES)]
    res = run_bass_kernel_spmd(nc, in_maps, core_ids=list(range(N_CORES)),
                               trace=_trace)
    outs = []
    for core in range(N_CORES):
        yc = res.results[core]["y"]          # [256 s_cat, 256 b]
        outs.append(np.ascontiguousarray(yc.T))
    full = np.concatenate(outs, axis=0)       # [2048, 256]
    if _trace:
        return full, res
    return full
